# revision 10
# baseline (speedup 1.0000x reference)
"""Trainium2 Bass kernel for nn_AttentionCellEncoder.

Contract: kernel(**inputs) takes FULL unsharded inputs (as produced by
setup_inputs) and returns the FULL [2048, 256] float32 output. Internally
shards cells across 8 NeuronCores (data-parallel over the cell dimension,
chunk_features table replicated), runs a Bass/Tile kernel via
run_bass_kernel_spmd, and reassembles the output.

All matmul datapaths run in bf16 (fp32 matmul costs 4 cycles/row on TRN2 vs
1 for bf16); accumulation stays fp32 in PSUM. Host-side the small weight
matrices are folded (attention in_proj into the q/k/v projections, out_proj
into the final projection) so the device does 3 GEMMs per token + attention.

Self-contained: all shapes/sharding hardcoded.
"""

import numpy as np

import concourse.bass as bass
import concourse.mybir as mybir
import concourse.tile as tile
from concourse import bacc
from concourse.bass_utils import run_bass_kernel_spmd
from concourse.masks import make_identity

FP = mybir.dt.float32
BF = mybir.dt.bfloat16
P = 128

# Problem dims
NUM_HEADS = 8
NUM_CHUNKS, INPUT_DIM = 50000, 768   # D = 768
HIDDEN_DIM, OUTPUT_DIM = 512, 256    # H = 512
NUM_CELLS, MAX_LEN = 2048, 64        # C, L
HEAD_DIM = HIDDEN_DIM // NUM_HEADS   # 64

N_CORES = 8
CELLS_PER_CORE = NUM_CELLS // N_CORES          # 256
TILES_PER_CORE = CELLS_PER_CORE // 2           # 128 tiles of 2 cells / 128 tokens
TILES_PER_BLOCK = 4                            # 512 tokens per block
BLOCKS = TILES_PER_CORE // TILES_PER_BLOCK     # 32
DCH = INPUT_DIM // P                           # 6 d-chunks
HCH = HIDDEN_DIM // P                          # 4 h-chunks
TOK_BLK = TILES_PER_BLOCK * P                  # 512
CELL_GROUPS = CELLS_PER_CORE // P              # 2 output groups of 128 cells


def build_kernel(with_v_bias: bool, repeat: int = 1):
    """Trace and compile the per-core SPMD kernel. Returns the Bass object."""
    nc = bacc.Bacc(None)

    table = nc.dram_tensor("table", [NUM_CHUNKS, INPUT_DIM], BF, kind="ExternalInput")
    wq_t = nc.dram_tensor("wq_t", [INPUT_DIM, HIDDEN_DIM], BF, kind="ExternalInput")
    wk_t = nc.dram_tensor("wk_t", [INPUT_DIM, HIDDEN_DIM], BF, kind="ExternalInput")
    wv_t = nc.dram_tensor("wv_t", [INPUT_DIM, HIDDEN_DIM], BF, kind="ExternalInput")
    wf_t = nc.dram_tensor("wf_t", [HIDDEN_DIM, OUTPUT_DIM], BF, kind="ExternalInput")
    bq_c = nc.dram_tensor("bq_c", [P, HCH], FP, kind="ExternalInput")
    bk_c = nc.dram_tensor("bk_c", [P, HCH], FP, kind="ExternalInput")
    bv_r = nc.dram_tensor("bv_r", [1, HIDDEN_DIM], BF, kind="ExternalInput")
    idx = nc.dram_tensor("idx", [CELLS_PER_CORE * MAX_LEN], mybir.dt.int32,
                         kind="ExternalInput")
    maskb = nc.dram_tensor("maskb", [CELLS_PER_CORE * MAX_LEN], FP,
                           kind="ExternalInput")
    u2 = nc.dram_tensor("u2", [TILES_PER_CORE * P, 2], BF, kind="ExternalInput")
    out = nc.dram_tensor("out", [CELLS_PER_CORE, OUTPUT_DIM], FP,
                         kind="ExternalOutput")

    with tile.TileContext(nc) as tc:
        with (
            tc.tile_pool(name="const", bufs=1) as cpool,
            tc.tile_pool(name="xp", bufs=3) as xpool,
            tc.tile_pool(name="blk", bufs=2) as bpool,
            tc.tile_pool(name="sm", bufs=3) as spool,
            tc.tile_pool(name="op", bufs=2) as opool,
            tc.tile_pool(name="ps", bufs=2, space="PSUM") as pspool,
        ):
            ident = cpool.tile([P, P], BF)
            make_identity(nc, ident[:])
            ident_f = cpool.tile([P, P], FP)
            make_identity(nc, ident_f[:])
            ones = cpool.tile([P, 1], BF)
            nc.gpsimd.memset(ones[:], 1.0)

            wq_sb = cpool.tile([P, DCH * HIDDEN_DIM], BF)
            wk_sb = cpool.tile([P, DCH * HIDDEN_DIM], BF)
            wv_sb = cpool.tile([P, DCH * HIDDEN_DIM], BF)
            for j in range(DCH):
                s = slice(j * HIDDEN_DIM, (j + 1) * HIDDEN_DIM)
                d = slice(j * P, (j + 1) * P)
                nc.sync.dma_start(out=wq_sb[:, s], in_=wq_t[d, :])
                nc.sync.dma_start(out=wk_sb[:, s], in_=wk_t[d, :])
                nc.sync.dma_start(out=wv_sb[:, s], in_=wv_t[d, :])
            wf_sb = cpool.tile([P, HCH * OUTPUT_DIM], BF)
            for c in range(HCH):
                nc.sync.dma_start(out=wf_sb[:, c * OUTPUT_DIM:(c + 1) * OUTPUT_DIM],
                                  in_=wf_t[c * P:(c + 1) * P, :])
            bq_sb = cpool.tile([P, HCH], FP)
            bk_sb = cpool.tile([P, HCH], FP)
            nc.sync.dma_start(out=bq_sb[:], in_=bq_c[:, :])
            nc.sync.dma_start(out=bk_sb[:], in_=bk_c[:, :])
            if with_v_bias:
                ones1 = cpool.tile([1, P], BF)
                nc.gpsimd.memset(ones1[:], 1.0)
                bv_sb = cpool.tile([1, HIDDEN_DIM], BF)
                nc.sync.dma_start(out=bv_sb[:], in_=bv_r[:, :])

            for rep in range(repeat):
                # pooled rows accumulate here per group of 128 cells (bf16 sbuf):
                # pooled[cell_local, :] = pooled_cell
                pooled = [None] * CELL_GROUPS

                for b in range(BLOCKS):
                    g = b // (BLOCKS // CELL_GROUPS)
                    if pooled[g] is None:
                        pooled[g] = opool.tile([P, HIDDEN_DIM], FP, tag="pooled",
                                               name=f"pooled{g}_{rep}", bufs=2)
                    # ---- gather + transpose: xT[:, j*512 + tok] = x^T ----
                    xT = bpool.tile([P, DCH * TOK_BLK], BF, tag="xT")
                    for t in range(TILES_PER_BLOCK):
                        row0 = (b * TILES_PER_BLOCK + t) * P
                        idx_sb = spool.tile([P, 1], mybir.dt.int32, tag="idx")
                        nc.sync.dma_start(out=idx_sb[:, :1],
                                          in_=idx[row0:row0 + P, None])
                        x = xpool.tile([P, INPUT_DIM], BF, tag="x")
                        nc.gpsimd.indirect_dma_start(
                            out=x[:], out_offset=None, in_=table[:],
                            in_offset=bass.IndirectOffsetOnAxis(ap=idx_sb[:, :1], axis=0),
                        )
                        pa = pspool.tile([P, INPUT_DIM], BF, tag="xp")
                        for j in range(DCH):
                            nc.tensor.transpose(out=pa[:, j * P:(j + 1) * P],
                                                in_=x[:, j * P:(j + 1) * P],
                                                identity=ident[:])
                        nc.vector.tensor_copy(
                            out=xT[:].rearrange("p (j n) -> p j n", j=DCH)
                                [:, :, t * P:(t + 1) * P],
                            in_=pa[:].rearrange("p (j n) -> p j n", j=DCH),
                        )

                    # ---- qT, kT: weight-stationary, N=512 tokens ----
                    # qT layout: [128 part = 2 heads x 64 d, HCH chunks x 512 tok]
                    # *_sw = partition halves swapped (for diagonal-tile scores)
                    qT = bpool.tile([P, HCH * TOK_BLK], BF, tag="qT")
                    kT = bpool.tile([P, HCH * TOK_BLK], BF, tag="kT")
                    qT_sw = bpool.tile([P, HCH * TOK_BLK], BF, tag="qTsw")
                    kT_sw = bpool.tile([P, HCH * TOK_BLK], BF, tag="kTsw")
                    for (wsb, bsb, dst, dsw) in ((wq_sb, bq_sb, qT, qT_sw),
                                                 (wk_sb, bk_sb, kT, kT_sw)):
                        for hc in range(HCH):
                            acc = pspool.tile([P, TOK_BLK], FP, tag="acc")
                            for j in range(DCH):
                                nc.tensor.matmul(
                                    out=acc[:],
                                    lhsT=wsb[:, j * HIDDEN_DIM + hc * P:
                                             j * HIDDEN_DIM + (hc + 1) * P],
                                    rhs=xT[:, j * TOK_BLK:(j + 1) * TOK_BLK],
                                    start=(j == 0), stop=(j == DCH - 1),
                                )
                            nc.scalar.activation(
                                out=dst[:, hc * TOK_BLK:(hc + 1) * TOK_BLK],
                                in_=acc[:],
                                func=mybir.ActivationFunctionType.Identity,
                                bias=bsb[:, hc:hc + 1])
                        nc.sync.dma_start(out=dsw[0:64, :], in_=dst[64:P, :])
                        nc.sync.dma_start(out=dsw[64:P, :], in_=dst[0:64, :])

                    # ---- v: x-stationary per tile, [128 tok, 512 h] ----
                    v = bpool.tile([P, TILES_PER_BLOCK * HIDDEN_DIM], BF, tag="v")
                    for t in range(TILES_PER_BLOCK):
                        acc = pspool.tile([P, HIDDEN_DIM], FP, tag="acc")
                        nmm = DCH + (1 if with_v_bias else 0)
                        for j in range(DCH):
                            nc.tensor.matmul(
                                out=acc[:],
                                lhsT=xT[:, j * TOK_BLK + t * P:j * TOK_BLK + (t + 1) * P],
                                rhs=wv_sb[:, j * HIDDEN_DIM:(j + 1) * HIDDEN_DIM],
                                start=(j == 0), stop=(j == nmm - 1),
                            )
                        if with_v_bias:
                            nc.tensor.matmul(out=acc[:], lhsT=ones1[0:1, :],
                                             rhs=bv_sb[0:1, :], start=False, stop=True)
                        nc.vector.tensor_copy(
                            out=v[:, t * HIDDEN_DIM:(t + 1) * HIDDEN_DIM], in_=acc[:])

                    # ---- attention per tile (2 cells) ----
                    for t in range(TILES_PER_BLOCK):
                        gt = b * TILES_PER_BLOCK + t      # global tile id
                        row0 = gt * P
                        mk = spool.tile([P, 1], FP, tag="mk")
                        nc.sync.dma_start(out=mk[:, :1], in_=maskb[row0:row0 + P, None])
                        u2_sb = spool.tile([P, 2], BF, tag="u2")
                        nc.sync.dma_start(out=u2_sb[:], in_=u2[row0:row0 + P, :])

                        # scores^T: [2c x 64 m, 8h x 64 l]; diagonal tiles only:
                        # head h data taken from the copy that has it at half c.
                        sc = pspool.tile([P, HIDDEN_DIM], FP, tag="att")
                        for h in range(NUM_HEADS):
                            hc = h // 2
                            for c in range(2):   # c inner: T0/T10 quads overlap
                                pr = slice(c * 64, c * 64 + 64)
                                kk, qq = (kT, qT) if h % 2 == c else (kT_sw, qT_sw)
                                fw = slice(hc * TOK_BLK + t * P + c * 64,
                                           hc * TOK_BLK + t * P + c * 64 + 64)
                                nc.tensor.matmul(
                                    out=sc[pr, h * 64:h * 64 + 64],
                                    lhsT=kk[pr, fw], rhs=qq[pr, fw],
                                    start=True, stop=True,
                                )
                        e = spool.tile([P, HIDDEN_DIM], BF, tag="e")
                        nc.scalar.activation(out=e[:], in_=sc[:],
                                             func=mybir.ActivationFunctionType.Exp,
                                             bias=mk[:, :1])

                        # ctx (unnormalized) + per-(l,h) denominators
                        ctx = pspool.tile([P, HIDDEN_DIM], FP, tag="att")
                        sden = pspool.tile([P, NUM_HEADS], FP, tag="att")
                        for h in range(NUM_HEADS):
                            for c in range(2):   # c inner: T0/T10 quads overlap
                                el = e[c * 64:c * 64 + 64, h * 64:h * 64 + 64]
                                nc.tensor.matmul(
                                    out=ctx[c * 64:c * 64 + 64, h * 64:h * 64 + 64],
                                    lhsT=el,
                                    rhs=v[c * 64:c * 64 + 64,
                                          t * HIDDEN_DIM + h * 64:
                                          t * HIDDEN_DIM + h * 64 + 64],
                                    start=True, stop=True,
                                )
                                nc.tensor.matmul(
                                    out=sden[c * 64:c * 64 + 64, h:h + 1],
                                    lhsT=el, rhs=ones[c * 64:c * 64 + 64, 0:1],
                                    start=True, stop=True,
                                )
                        r = spool.tile([P, NUM_HEADS], FP, tag="r")
                        nc.vector.reciprocal(out=r[:], in_=sden[:])
                        cn = spool.tile([P, HIDDEN_DIM], BF, tag="cn")
                        nc.vector.tensor_tensor(
                            out=cn[:].rearrange("p (h d) -> p h d", h=NUM_HEADS),
                            in0=ctx[:].rearrange("p (h d) -> p h d", h=NUM_HEADS),
                            in1=r[:, :, None].to_broadcast([P, NUM_HEADS, HEAD_DIM]),
                            op=mybir.AluOpType.mult,
                        )
                        # pooled rows: pooled[2 cells of this tile, :] =
                        #   u2^T @ cn   (u2 col c is zero outside cell c's rows)
                        tl = gt - g * (TILES_PER_CORE // CELL_GROUPS)
                        bp = pspool.tile([2, HIDDEN_DIM], FP, tag="bp")
                        nc.tensor.matmul(
                            out=bp[0:2, :],
                            lhsT=u2_sb[:, 0:2], rhs=cn[:],
                            start=True, stop=True,
                        )
                        bp_sb = spool.tile([2, HIDDEN_DIM], FP, tag="bps")
                        nc.vector.tensor_copy(out=bp_sb[0:2, :], in_=bp[0:2, :])
                        nc.sync.dma_start(
                            out=pooled[g][2 * tl:2 * tl + 2, :], in_=bp_sb[0:2, :])

                # ---- final projection per group of 128 cells ----
                for g in range(CELL_GROUPS):
                    pt = pspool.tile([P, HIDDEN_DIM], FP, tag="xp")
                    for hc in range(HCH):
                        nc.tensor.transpose(out=pt[:, hc * P:(hc + 1) * P],
                                            in_=pooled[g][:, hc * P:(hc + 1) * P],
                                            identity=ident_f[:])
                    ptsb = opool.tile([P, HIDDEN_DIM], BF, tag="ptsb")
                    nc.vector.tensor_copy(out=ptsb[:], in_=pt[:])
                    acc = pspool.tile([P, OUTPUT_DIM], FP, tag="acc")
                    for hc in range(HCH):
                        nc.tensor.matmul(
                            out=acc[:], lhsT=ptsb[:, hc * P:(hc + 1) * P],
                            rhs=wf_sb[:, hc * OUTPUT_DIM:(hc + 1) * OUTPUT_DIM],
                            start=(hc == 0), stop=(hc == HCH - 1),
                        )
                    osb = opool.tile([P, OUTPUT_DIM], FP, tag="osb")
                    nc.scalar.activation(out=osb[:], in_=acc[:],
                                         func=mybir.ActivationFunctionType.Copy)
                    nc.sync.dma_start(out=out[g * P:(g + 1) * P, :], in_=osb[:])

    nc.compile()
    return nc


def preprocess(chunk_features, Wq, bq, Wk, bk, Wv, bv, W_in, b_in, Wo, bo,
               Wout, bout, cell_idx, cell_len):
    """Host-side weight folding + per-core input maps. Returns (in_maps, b_final,
    with_v_bias)."""
    import ml_dtypes
    f32 = np.float32
    bf16 = ml_dtypes.bfloat16
    cf = np.ascontiguousarray(np.asarray(chunk_features, f32).astype(bf16))
    Wq, Wk, Wv = (np.asarray(w, f32) for w in (Wq, Wk, Wv))
    bq, bk, bv = (np.asarray(x, f32) for x in (bq, bk, bv))
    W_in = np.asarray(W_in, f32)
    b_in = np.asarray(b_in, f32)
    Wo, bo = np.asarray(Wo, f32), np.asarray(bo, f32)
    Wout, bout = np.asarray(Wout, f32), np.asarray(bout, f32)

    Wiq, Wik, Wiv = np.split(W_in, 3, axis=0)
    biq, bik, biv = np.split(b_in, 3)
    scale = f32(1.0 / np.sqrt(HEAD_DIM))
    wq_eff = (Wiq @ Wq) * scale          # [512, 768]
    wk_eff = Wik @ Wk
    wv_eff = Wiv @ Wv
    bq_eff = (Wiq @ bq + biq) * scale    # [512]
    bk_eff = Wik @ bk + bik
    bv_eff = Wiv @ bv + biv
    wfin = Wout @ Wo                     # [256, 512]
    b_final = bo @ Wout.T + bout         # [256]

    wq_t = np.ascontiguousarray(wq_eff.T.astype(bf16))   # [768, 512]
    wk_t = np.ascontiguousarray(wk_eff.T.astype(bf16))
    wv_t = np.ascontiguousarray(wv_eff.T.astype(bf16))
    wf_t = np.ascontiguousarray(wfin.T.astype(bf16))     # [512, 256]
    bq_c = np.ascontiguousarray(bq_eff.reshape(HCH, P).T)  # [128, 4] f32
    bk_c = np.ascontiguousarray(bk_eff.reshape(HCH, P).T)
    bv_r = np.ascontiguousarray(bv_eff.reshape(1, HIDDEN_DIM).astype(bf16))
    with_v_bias = bool(np.any(bv_eff != 0))

    ci = np.asarray(cell_idx).astype(np.int32)             # [2048, 64]
    ln = np.maximum(np.asarray(cell_len).astype(np.int64), 1)
    ln = np.minimum(ln, MAX_LEN).astype(np.int32)          # [2048]
    pos = np.arange(MAX_LEN, dtype=np.int32)
    valid = pos[None, :] < ln[:, None]                     # [2048, 64]
    maskb_full = np.where(valid, f32(0.0), f32(-1e30))     # [2048, 64]
    u_full = (valid / ln[:, None]).astype(f32)             # [2048, 64]

    in_maps = []
    for core in range(N_CORES):
        cs = slice(core * CELLS_PER_CORE, (core + 1) * CELLS_PER_CORE)
        idx_c = np.ascontiguousarray(ci[cs].reshape(-1))
        mb_c = np.ascontiguousarray(maskb_full[cs].reshape(-1))
        u_c = u_full[cs]                                   # [256, 64]
        u2_c = np.zeros((TILES_PER_CORE, P, 2), f32)
        u2_c[:, 0:64, 0] = u_c[0::2]
        u2_c[:, 64:128, 1] = u_c[1::2]
        in_maps.append({
            "table": cf,
            "wq_t": wq_t, "wk_t": wk_t, "wv_t": wv_t, "wf_t": wf_t,
            "bq_c": bq_c, "bk_c": bk_c, "bv_r": bv_r,
            "idx": idx_c, "maskb": mb_c,
            "u2": u2_c.reshape(TILES_PER_CORE * P, 2).astype(bf16),
        })
    return in_maps, b_final, with_v_bias


_NC_CACHE: dict = {}


def get_nc(with_v_bias: bool, repeat: int = 1):
    key = (with_v_bias, repeat)
    if key not in _NC_CACHE:
        _NC_CACHE[key] = build_kernel(with_v_bias, repeat=repeat)
    return _NC_CACHE[key]


def kernel(**inputs) -> np.ndarray:
    in_maps, b_final, with_v_bias = preprocess(**inputs)
    nc = get_nc(with_v_bias)
    res = run_bass_kernel_spmd(nc, in_maps, list(range(N_CORES)))
    out = np.concatenate([res.results[i]["out"] for i in range(N_CORES)], axis=0)
    return (out + b_final[None, :]).astype(np.float32)


# revision 15
# speedup vs baseline: 1.0306x; 1.0306x over previous
"""Trainium2 Bass kernel for nn_AttentionCellEncoder.

Contract: kernel(**inputs) takes FULL unsharded inputs (as produced by
setup_inputs) and returns the FULL [2048, 256] float32 output. Internally
shards cells across 8 NeuronCores (data-parallel over the cell dimension,
chunk_features table replicated), runs a Bass/Tile kernel via
run_bass_kernel_spmd, and reassembles the output.

All matmul datapaths run in bf16 (fp32 matmul costs 4 cycles/row on TRN2 vs
1 for bf16); accumulation stays fp32 in PSUM. Host-side the small weight
matrices are folded (attention in_proj into the q/k/v projections, out_proj
into the final projection) so the device does 3 GEMMs per token + attention.

Self-contained: all shapes/sharding hardcoded.
"""

import numpy as np

import concourse.bass as bass
import concourse.mybir as mybir
import concourse.tile as tile
from concourse import bacc
from concourse.bass_utils import run_bass_kernel_spmd
from concourse.masks import make_identity

FP = mybir.dt.float32
BF = mybir.dt.bfloat16
P = 128

# Problem dims
NUM_HEADS = 8
NUM_CHUNKS, INPUT_DIM = 50000, 768   # D = 768
HIDDEN_DIM, OUTPUT_DIM = 512, 256    # H = 512
NUM_CELLS, MAX_LEN = 2048, 64        # C, L
HEAD_DIM = HIDDEN_DIM // NUM_HEADS   # 64

N_CORES = 8
CELLS_PER_CORE = NUM_CELLS // N_CORES          # 256
TILES_PER_CORE = CELLS_PER_CORE // 2           # 128 tiles of 2 cells / 128 tokens
TILES_PER_BLOCK = 4                            # 512 tokens per block
BLOCKS = TILES_PER_CORE // TILES_PER_BLOCK     # 32
DCH = INPUT_DIM // P                           # 6 d-chunks
HCH = HIDDEN_DIM // P                          # 4 h-chunks
TOK_BLK = TILES_PER_BLOCK * P                  # 512
CELL_GROUPS = CELLS_PER_CORE // P              # 2 output groups of 128 cells


def build_kernel(with_v_bias: bool, repeat: int = 1):
    """Trace and compile the per-core SPMD kernel. Returns the Bass object."""
    nc = bacc.Bacc(None)

    table = nc.dram_tensor("table", [NUM_CHUNKS, INPUT_DIM], BF, kind="ExternalInput")
    wq_t = nc.dram_tensor("wq_t", [INPUT_DIM, HIDDEN_DIM], BF, kind="ExternalInput")
    wk_t = nc.dram_tensor("wk_t", [INPUT_DIM, HIDDEN_DIM], BF, kind="ExternalInput")
    wv_t = nc.dram_tensor("wv_t", [INPUT_DIM, HIDDEN_DIM], BF, kind="ExternalInput")
    wf_t = nc.dram_tensor("wf_t", [HIDDEN_DIM, OUTPUT_DIM], BF, kind="ExternalInput")
    bq_c = nc.dram_tensor("bq_c", [P, HCH], FP, kind="ExternalInput")
    bk_c = nc.dram_tensor("bk_c", [P, HCH], FP, kind="ExternalInput")
    bv_r = nc.dram_tensor("bv_r", [1, HIDDEN_DIM], BF, kind="ExternalInput")
    idx = nc.dram_tensor("idx", [CELLS_PER_CORE * MAX_LEN], mybir.dt.int32,
                         kind="ExternalInput")
    maskb = nc.dram_tensor("maskb", [CELLS_PER_CORE * MAX_LEN], FP,
                           kind="ExternalInput")
    u2 = nc.dram_tensor("u2", [TILES_PER_CORE * P, 2], BF, kind="ExternalInput")
    out = nc.dram_tensor("out", [CELLS_PER_CORE, OUTPUT_DIM], FP,
                         kind="ExternalOutput")

    with tile.TileContext(nc) as tc:
        with (
            tc.tile_pool(name="const", bufs=1) as cpool,
            tc.tile_pool(name="xp", bufs=3) as xpool,
            tc.tile_pool(name="blk", bufs=2) as bpool,
            tc.tile_pool(name="sm", bufs=3) as spool,
            tc.tile_pool(name="op", bufs=2) as opool,
            tc.tile_pool(name="ps", bufs=2, space="PSUM") as pspool,
        ):
            ident = cpool.tile([P, P], BF)
            make_identity(nc, ident[:])
            ident_f = cpool.tile([P, P], FP)
            make_identity(nc, ident_f[:])

            wq_sb = cpool.tile([P, DCH * HIDDEN_DIM], BF)
            wk_sb = cpool.tile([P, DCH * HIDDEN_DIM], BF)
            wv_sb = cpool.tile([P, DCH * HIDDEN_DIM], BF)
            for j in range(DCH):
                s = slice(j * HIDDEN_DIM, (j + 1) * HIDDEN_DIM)
                d = slice(j * P, (j + 1) * P)
                nc.sync.dma_start(out=wq_sb[:, s], in_=wq_t[d, :])
                nc.sync.dma_start(out=wk_sb[:, s], in_=wk_t[d, :])
                nc.sync.dma_start(out=wv_sb[:, s], in_=wv_t[d, :])
            wf_sb = cpool.tile([P, HCH * OUTPUT_DIM], BF)
            for c in range(HCH):
                nc.sync.dma_start(out=wf_sb[:, c * OUTPUT_DIM:(c + 1) * OUTPUT_DIM],
                                  in_=wf_t[c * P:(c + 1) * P, :])
            bq_sb = cpool.tile([P, HCH], FP)
            bk_sb = cpool.tile([P, HCH], FP)
            nc.sync.dma_start(out=bq_sb[:], in_=bq_c[:, :])
            nc.sync.dma_start(out=bk_sb[:], in_=bk_c[:, :])
            if with_v_bias:
                ones1 = cpool.tile([1, P], BF)
                nc.gpsimd.memset(ones1[:], 1.0)
                bv_sb = cpool.tile([1, HIDDEN_DIM], BF)
                nc.sync.dma_start(out=bv_sb[:], in_=bv_r[:, :])

            for rep in range(repeat):
                # pooled rows accumulate here per group of 128 cells (bf16 sbuf):
                # pooled[cell_local, :] = pooled_cell
                pooled = [None] * CELL_GROUPS

                for b in range(BLOCKS):
                    g = b // (BLOCKS // CELL_GROUPS)
                    if pooled[g] is None:
                        pooled[g] = opool.tile([P, HIDDEN_DIM], FP, tag="pooled",
                                               name=f"pooled{g}_{rep}", bufs=2)
                    # ---- gather + transpose: xT[:, j*512 + tok] = x^T ----
                    xT = bpool.tile([P, DCH * TOK_BLK], BF, tag="xT")
                    for t in range(TILES_PER_BLOCK):
                        row0 = (b * TILES_PER_BLOCK + t) * P
                        idx_sb = spool.tile([P, 1], mybir.dt.int32, tag="idx")
                        nc.sync.dma_start(out=idx_sb[:, :1],
                                          in_=idx[row0:row0 + P, None])
                        x = xpool.tile([P, INPUT_DIM], BF, tag="x")
                        nc.gpsimd.indirect_dma_start(
                            out=x[:], out_offset=None, in_=table[:],
                            in_offset=bass.IndirectOffsetOnAxis(ap=idx_sb[:, :1], axis=0),
                        )
                        pa = pspool.tile([P, INPUT_DIM], BF, tag="xp")
                        for j in range(DCH):
                            nc.tensor.transpose(out=pa[:, j * P:(j + 1) * P],
                                                in_=x[:, j * P:(j + 1) * P],
                                                identity=ident[:])
                        nc.vector.tensor_copy(
                            out=xT[:].rearrange("p (j n) -> p j n", j=DCH)
                                [:, :, t * P:(t + 1) * P],
                            in_=pa[:].rearrange("p (j n) -> p j n", j=DCH),
                        )

                    # ---- qT, kT: weight-stationary, N=512 tokens ----
                    # qT layout: [128 part = 2 heads x 64 d, HCH chunks x 512 tok]
                    # *_sw = partition halves swapped (for diagonal-tile scores)
                    qT = bpool.tile([P, HCH * TOK_BLK], BF, tag="qT")
                    kT = bpool.tile([P, HCH * TOK_BLK], BF, tag="kT")
                    qT_sw = bpool.tile([P, HCH * TOK_BLK], BF, tag="qTsw")
                    kT_sw = bpool.tile([P, HCH * TOK_BLK], BF, tag="kTsw")
                    for (wsb, bsb, dst, dsw) in ((wq_sb, bq_sb, qT, qT_sw),
                                                 (wk_sb, bk_sb, kT, kT_sw)):
                        for hc in range(HCH):
                            acc = pspool.tile([P, TOK_BLK], FP, tag="acc")
                            for j in range(DCH):
                                nc.tensor.matmul(
                                    out=acc[:],
                                    lhsT=wsb[:, j * HIDDEN_DIM + hc * P:
                                             j * HIDDEN_DIM + (hc + 1) * P],
                                    rhs=xT[:, j * TOK_BLK:(j + 1) * TOK_BLK],
                                    start=(j == 0), stop=(j == DCH - 1),
                                )
                            nc.scalar.activation(
                                out=dst[:, hc * TOK_BLK:(hc + 1) * TOK_BLK],
                                in_=acc[:],
                                func=mybir.ActivationFunctionType.Identity,
                                bias=bsb[:, hc:hc + 1])
                        nc.sync.dma_start(out=dsw[0:64, :], in_=dst[64:P, :])
                        nc.sync.dma_start(out=dsw[64:P, :], in_=dst[0:64, :])

                    # ---- v: x-stationary per tile; layout [128 tok, t, h, 72]
                    # with a ones column at [..., 64] so the ctx matmul (N=65)
                    # also produces the softmax denominator ----
                    v = bpool.tile([P, TILES_PER_BLOCK * NUM_HEADS * 72], BF,
                                   tag="v")
                    v4 = v[:].rearrange("p (t h e) -> p t h e",
                                        t=TILES_PER_BLOCK, h=NUM_HEADS)
                    nc.gpsimd.memset(v4[:, :, :, 64:65], 1.0)
                    for t in range(TILES_PER_BLOCK):
                        acc = pspool.tile([P, HIDDEN_DIM], FP, tag="acc")
                        nmm = DCH + (1 if with_v_bias else 0)
                        for j in range(DCH):
                            nc.tensor.matmul(
                                out=acc[:],
                                lhsT=xT[:, j * TOK_BLK + t * P:j * TOK_BLK + (t + 1) * P],
                                rhs=wv_sb[:, j * HIDDEN_DIM:(j + 1) * HIDDEN_DIM],
                                start=(j == 0), stop=(j == nmm - 1),
                            )
                        if with_v_bias:
                            nc.tensor.matmul(out=acc[:], lhsT=ones1[0:1, :],
                                             rhs=bv_sb[0:1, :], start=False, stop=True)
                        nc.vector.tensor_copy(
                            out=v4[:, t, :, 0:64],
                            in_=acc[:].rearrange("p (h d) -> p h d", h=NUM_HEADS))

                    # ---- attention per tile (2 cells) ----
                    cns = []
                    for t in range(TILES_PER_BLOCK):
                        gt = b * TILES_PER_BLOCK + t      # global tile id
                        row0 = gt * P
                        mk = spool.tile([P, 1], FP, tag="mk")
                        nc.sync.dma_start(out=mk[:, :1], in_=maskb[row0:row0 + P, None])
                        u2_sb = spool.tile([P, 2], BF, tag="u2", bufs=5)
                        nc.sync.dma_start(out=u2_sb[:], in_=u2[row0:row0 + P, :])

                        # scores^T: [2c x 64 m, 8h x 64 l]; diagonal tiles only:
                        # head h data taken from the copy that has it at half c.
                        sc = pspool.tile([P, HIDDEN_DIM], FP, tag="att")
                        for h in range(NUM_HEADS):
                            hc = h // 2
                            for c in range(2):   # c inner: T0/T10 quads overlap
                                pr = slice(c * 64, c * 64 + 64)
                                kk, qq = (kT, qT) if h % 2 == c else (kT_sw, qT_sw)
                                fw = slice(hc * TOK_BLK + t * P + c * 64,
                                           hc * TOK_BLK + t * P + c * 64 + 64)
                                nc.tensor.matmul(
                                    out=sc[pr, h * 64:h * 64 + 64],
                                    lhsT=kk[pr, fw], rhs=qq[pr, fw],
                                    start=True, stop=True,
                                )
                        e = spool.tile([P, HIDDEN_DIM], BF, tag="e")
                        nc.scalar.activation(out=e[:], in_=sc[:],
                                             func=mybir.ActivationFunctionType.Exp,
                                             bias=mk[:, :1])

                        # ctx (unnormalized) + denominators fused, 4 heads per
                        # psum bank: rhs is [v_h | ones], so column 64 of each
                        # 72-wide head block is sum_m e[m, l]. Half A
                        # normalizes on DVE while half B's matmuls run.
                        cn = spool.tile([P, HIDDEN_DIM], BF, tag="cn", bufs=5)
                        for half in range(2):
                            cdh = pspool.tile([P, 512], FP, tag="cd",
                                              name=f"cd{half}")
                            for hj in range(4):
                                h = half * 4 + hj
                                for c in range(2):
                                    el = e[c * 64:c * 64 + 64, h * 64:h * 64 + 64]
                                    col = 72 * hj
                                    nc.tensor.matmul(
                                        out=cdh[c * 64:c * 64 + 64, col:col + 65],
                                        lhsT=el,
                                        rhs=v4[c * 64:c * 64 + 64, t, h, 0:65],
                                        start=True, stop=True,
                                    )
                            cdv = cdh[:, 0:288].rearrange("p (j s) -> p j s", j=4)
                            r = spool.tile([P, 4], FP, tag="r")
                            nc.vector.reciprocal(out=r[:], in_=cdv[:, :, 64])
                            nc.vector.tensor_tensor(
                                out=cn[:, half * 256:(half + 1) * 256]
                                    .rearrange("p (j d) -> p j d", j=4),
                                in0=cdv[:, :, 0:64],
                                in1=r[:, :, None].to_broadcast([P, 4, HEAD_DIM]),
                                op=mybir.AluOpType.mult,
                            )
                        cns.append((gt, cn, u2_sb))

                    # pooled rows after the block (keeps PE off the DVE
                    # normalize critical path): pooled[2 cells, :] = u2^T @ cn
                    for (gt, cn, u2_sb) in cns:
                        tl = gt - g * (TILES_PER_CORE // CELL_GROUPS)
                        bp = pspool.tile([2, HIDDEN_DIM], FP, tag="att")
                        nc.tensor.matmul(
                            out=bp[0:2, :],
                            lhsT=u2_sb[:, 0:2], rhs=cn[:],
                            start=True, stop=True,
                        )
                        bp_sb = spool.tile([2, HIDDEN_DIM], FP, tag="bps")
                        nc.vector.tensor_copy(out=bp_sb[0:2, :], in_=bp[0:2, :])
                        nc.sync.dma_start(
                            out=pooled[g][2 * tl:2 * tl + 2, :], in_=bp_sb[0:2, :])

                # ---- final projection per group of 128 cells ----
                for g in range(CELL_GROUPS):
                    pt = pspool.tile([P, HIDDEN_DIM], FP, tag="xp")
                    for hc in range(HCH):
                        nc.tensor.transpose(out=pt[:, hc * P:(hc + 1) * P],
                                            in_=pooled[g][:, hc * P:(hc + 1) * P],
                                            identity=ident_f[:])
                    ptsb = opool.tile([P, HIDDEN_DIM], BF, tag="ptsb")
                    nc.vector.tensor_copy(out=ptsb[:], in_=pt[:])
                    acc = pspool.tile([P, OUTPUT_DIM], FP, tag="acc")
                    for hc in range(HCH):
                        nc.tensor.matmul(
                            out=acc[:], lhsT=ptsb[:, hc * P:(hc + 1) * P],
                            rhs=wf_sb[:, hc * OUTPUT_DIM:(hc + 1) * OUTPUT_DIM],
                            start=(hc == 0), stop=(hc == HCH - 1),
                        )
                    osb = opool.tile([P, OUTPUT_DIM], FP, tag="osb")
                    nc.scalar.activation(out=osb[:], in_=acc[:],
                                         func=mybir.ActivationFunctionType.Copy)
                    nc.sync.dma_start(out=out[g * P:(g + 1) * P, :], in_=osb[:])

    nc.compile()
    return nc


def preprocess(chunk_features, Wq, bq, Wk, bk, Wv, bv, W_in, b_in, Wo, bo,
               Wout, bout, cell_idx, cell_len):
    """Host-side weight folding + per-core input maps. Returns (in_maps, b_final,
    with_v_bias)."""
    import ml_dtypes
    f32 = np.float32
    bf16 = ml_dtypes.bfloat16
    cf = np.ascontiguousarray(np.asarray(chunk_features, f32).astype(bf16))
    Wq, Wk, Wv = (np.asarray(w, f32) for w in (Wq, Wk, Wv))
    bq, bk, bv = (np.asarray(x, f32) for x in (bq, bk, bv))
    W_in = np.asarray(W_in, f32)
    b_in = np.asarray(b_in, f32)
    Wo, bo = np.asarray(Wo, f32), np.asarray(bo, f32)
    Wout, bout = np.asarray(Wout, f32), np.asarray(bout, f32)

    Wiq, Wik, Wiv = np.split(W_in, 3, axis=0)
    biq, bik, biv = np.split(b_in, 3)
    scale = f32(1.0 / np.sqrt(HEAD_DIM))
    wq_eff = (Wiq @ Wq) * scale          # [512, 768]
    wk_eff = Wik @ Wk
    wv_eff = Wiv @ Wv
    bq_eff = (Wiq @ bq + biq) * scale    # [512]
    bk_eff = Wik @ bk + bik
    bv_eff = Wiv @ bv + biv
    wfin = Wout @ Wo                     # [256, 512]
    b_final = bo @ Wout.T + bout         # [256]

    wq_t = np.ascontiguousarray(wq_eff.T.astype(bf16))   # [768, 512]
    wk_t = np.ascontiguousarray(wk_eff.T.astype(bf16))
    wv_t = np.ascontiguousarray(wv_eff.T.astype(bf16))
    wf_t = np.ascontiguousarray(wfin.T.astype(bf16))     # [512, 256]
    bq_c = np.ascontiguousarray(bq_eff.reshape(HCH, P).T)  # [128, 4] f32
    bk_c = np.ascontiguousarray(bk_eff.reshape(HCH, P).T)
    bv_r = np.ascontiguousarray(bv_eff.reshape(1, HIDDEN_DIM).astype(bf16))
    with_v_bias = bool(np.any(bv_eff != 0))

    ci = np.asarray(cell_idx).astype(np.int32)             # [2048, 64]
    ln = np.maximum(np.asarray(cell_len).astype(np.int64), 1)
    ln = np.minimum(ln, MAX_LEN).astype(np.int32)          # [2048]
    pos = np.arange(MAX_LEN, dtype=np.int32)
    valid = pos[None, :] < ln[:, None]                     # [2048, 64]
    maskb_full = np.where(valid, f32(0.0), f32(-1e30))     # [2048, 64]
    u_full = (valid / ln[:, None]).astype(f32)             # [2048, 64]

    in_maps = []
    for core in range(N_CORES):
        cs = slice(core * CELLS_PER_CORE, (core + 1) * CELLS_PER_CORE)
        idx_c = np.ascontiguousarray(ci[cs].reshape(-1))
        mb_c = np.ascontiguousarray(maskb_full[cs].reshape(-1))
        u_c = u_full[cs]                                   # [256, 64]
        u2_c = np.zeros((TILES_PER_CORE, P, 2), f32)
        u2_c[:, 0:64, 0] = u_c[0::2]
        u2_c[:, 64:128, 1] = u_c[1::2]
        in_maps.append({
            "table": cf,
            "wq_t": wq_t, "wk_t": wk_t, "wv_t": wv_t, "wf_t": wf_t,
            "bq_c": bq_c, "bk_c": bk_c, "bv_r": bv_r,
            "idx": idx_c, "maskb": mb_c,
            "u2": u2_c.reshape(TILES_PER_CORE * P, 2).astype(bf16),
        })
    return in_maps, b_final, with_v_bias


_NC_CACHE: dict = {}


def get_nc(with_v_bias: bool, repeat: int = 1):
    key = (with_v_bias, repeat)
    if key not in _NC_CACHE:
        _NC_CACHE[key] = build_kernel(with_v_bias, repeat=repeat)
    return _NC_CACHE[key]


def kernel(**inputs) -> np.ndarray:
    in_maps, b_final, with_v_bias = preprocess(**inputs)
    nc = get_nc(with_v_bias)
    res = run_bass_kernel_spmd(nc, in_maps, list(range(N_CORES)))
    out = np.concatenate([res.results[i]["out"] for i in range(N_CORES)], axis=0)
    return (out + b_final[None, :]).astype(np.float32)


# revision 54
# speedup vs baseline: 1.9842x; 1.9253x over previous
"""Trainium2 Bass kernel for nn_AttentionCellEncoder.

Contract: kernel(**inputs) takes FULL unsharded inputs (as produced by
setup_inputs) and returns the FULL [2048, 256] float32 output. Internally
shards cells across 8 NeuronCores (data-parallel over the cell dimension,
chunk_features table replicated), runs a Bass/Tile kernel via
run_bass_kernel_spmd, and reassembles the output.

Datapath dtypes: the q/k projections run in fp8-e4m3 with DoubleRow perf
mode (0.5 cycles/row, 256-deep contraction per instruction; weights scaled
x64 to clear the fp8 subnormal floor, undone exactly by the psum-drain
activation's scale). Everything else runs in bf16 (fp32 matmul costs 4
cycles/row on TRN2 vs 1 for bf16); accumulation stays fp32 in PSUM. The
softmax denominator comes free from the ctx matmul via a ones column
appended to v (N=65). Host-side the small weight matrices are folded
(attention in_proj into the q/k/v projections, out_proj into the final
projection) so the device does 3 GEMMs per token + attention + pooling.

Self-contained: all shapes/sharding hardcoded.
"""

import numpy as np

import concourse.bass as bass
import concourse.mybir as mybir
import concourse.tile as tile
from concourse import bacc
from concourse.bass_utils import run_bass_kernel_spmd
from concourse.masks import make_identity

FP = mybir.dt.float32
BF = mybir.dt.bfloat16
F8 = mybir.dt.float8e4
P = 128
W8_SCALE = 64.0   # fp8 weights are stored x64 (subnormal floor), undone by
                  # the activation drain's scale=1/64

# True: mask invalid keys by zeroing v rows + the denominator ones-column,
# enabling one full-128-partition ctx matmul per head (exp split per cell
# half). False: classic -1e30 exp-bias mask with per-cell-half ctx matmuls.
FULL_CTX = False

# Problem dims
NUM_HEADS = 8
NUM_CHUNKS, INPUT_DIM = 50000, 768   # D = 768
HIDDEN_DIM, OUTPUT_DIM = 512, 256    # H = 512
NUM_CELLS, MAX_LEN = 2048, 64        # C, L
HEAD_DIM = HIDDEN_DIM // NUM_HEADS   # 64

N_CORES = 8
CELLS_PER_CORE = NUM_CELLS // N_CORES          # 256
TILES_PER_CORE = CELLS_PER_CORE // 2           # 128 tiles of 2 cells / 128 tokens
TILES_PER_BLOCK = 4                            # 512 tokens per block
BLOCKS = TILES_PER_CORE // TILES_PER_BLOCK     # 32
DCH = INPUT_DIM // P                           # 6 d-chunks
HCH = HIDDEN_DIM // P                          # 4 h-chunks
TOK_BLK = TILES_PER_BLOCK * P                  # 512
CELL_GROUPS = CELLS_PER_CORE // P              # 2 output groups of 128 cells


def build_kernel(with_v_bias: bool, repeat: int = 1, *, xp_bufs: int = 2,
                 acc_bufs: int = 2, att_bufs: int = 2, cd_bufs: int = 2):
    """Trace and compile the per-core SPMD kernel. Returns the Bass object."""
    nc = bacc.Bacc(None)

    table = nc.dram_tensor("table", [NUM_CHUNKS, INPUT_DIM], BF, kind="ExternalInput")
    wq_t = nc.dram_tensor("wq_t", [INPUT_DIM, HIDDEN_DIM], F8, kind="ExternalInput")
    wk_t = nc.dram_tensor("wk_t", [INPUT_DIM, HIDDEN_DIM], F8, kind="ExternalInput")
    wv_t = nc.dram_tensor("wv_t", [INPUT_DIM, HIDDEN_DIM], BF, kind="ExternalInput")
    wf_t = nc.dram_tensor("wf_t", [HIDDEN_DIM, OUTPUT_DIM], BF, kind="ExternalInput")
    bq_c = nc.dram_tensor("bq_c", [P, HCH], FP, kind="ExternalInput")
    bk_c = nc.dram_tensor("bk_c", [P, HCH], FP, kind="ExternalInput")
    bv_r = nc.dram_tensor("bv_r", [1, HIDDEN_DIM], BF, kind="ExternalInput")
    idx = nc.dram_tensor("idx", [CELLS_PER_CORE * MAX_LEN], mybir.dt.int32,
                         kind="ExternalInput")
    # 1.0 for valid token positions, 0.0 for padding (zeroes v rows + the
    # denominator ones-column, which masks those keys out of the softmax)
    maskb = nc.dram_tensor("maskb", [CELLS_PER_CORE * MAX_LEN], FP,
                           kind="ExternalInput")
    u2 = nc.dram_tensor("u2", [TILES_PER_CORE * P, 2], BF, kind="ExternalInput")
    out = nc.dram_tensor("out", [CELLS_PER_CORE, OUTPUT_DIM], FP,
                         kind="ExternalOutput")

    with tile.TileContext(nc) as tc:
        with (
            tc.tile_pool(name="const", bufs=1) as cpool,
            tc.tile_pool(name="xp", bufs=3) as xpool,
            tc.tile_pool(name="blk", bufs=2) as bpool,
            tc.tile_pool(name="sm", bufs=3) as spool,
            tc.tile_pool(name="op", bufs=2) as opool,
            tc.tile_pool(name="ps", bufs=2, space="PSUM") as pspool,
        ):
            ident = cpool.tile([P, P], BF)
            make_identity(nc, ident[:])
            ident_f = cpool.tile([P, P], FP)
            make_identity(nc, ident_f[:])

            wq_sb = cpool.tile([P, DCH * HIDDEN_DIM], F8)
            wk_sb = cpool.tile([P, DCH * HIDDEN_DIM], F8)
            wv_sb = cpool.tile([P, DCH * HIDDEN_DIM], BF)
            for j in range(DCH):
                s = slice(j * HIDDEN_DIM, (j + 1) * HIDDEN_DIM)
                d = slice(j * P, (j + 1) * P)
                nc.sync.dma_start(out=wq_sb[:, s], in_=wq_t[d, :])
                nc.sync.dma_start(out=wk_sb[:, s], in_=wk_t[d, :])
                nc.sync.dma_start(out=wv_sb[:, s], in_=wv_t[d, :])
            wf_sb = cpool.tile([P, HCH * OUTPUT_DIM], BF)
            for c in range(HCH):
                nc.sync.dma_start(out=wf_sb[:, c * OUTPUT_DIM:(c + 1) * OUTPUT_DIM],
                                  in_=wf_t[c * P:(c + 1) * P, :])
            bq_sb = cpool.tile([P, HCH], FP)
            bk_sb = cpool.tile([P, HCH], FP)
            nc.sync.dma_start(out=bq_sb[:], in_=bq_c[:, :])
            nc.sync.dma_start(out=bk_sb[:], in_=bk_c[:, :])
            if with_v_bias:
                ones1 = cpool.tile([1, P], BF)
                nc.gpsimd.memset(ones1[:], 1.0)
                bv_sb = cpool.tile([1, HIDDEN_DIM], BF)
                nc.sync.dma_start(out=bv_sb[:], in_=bv_r[:, :])

            # manual 3-deep ring of exp buffers [m, h, cell, l]; the
            # cross-cell blocks (key cell != query cell) are zeroed once and
            # never written, so ctx can contract over all 128 partitions
            e_bufs = []
            if FULL_CTX:
                for i in range(3):
                    eb = cpool.tile([P, NUM_HEADS, 2, 64], BF, name=f"ebuf{i}")
                    nc.gpsimd.memset(eb[0:64, :, 1, :], 0.0)
                    nc.gpsimd.memset(eb[64:P, :, 0, :], 0.0)
                    e_bufs.append(eb)

            for rep in range(repeat):
                # pooled rows accumulate here per group of 128 cells (bf16 sbuf):
                # pooled[cell_local, :] = pooled_cell
                pooled = [None] * CELL_GROUPS

                for b in range(BLOCKS):
                    g = b // (BLOCKS // CELL_GROUPS)
                    if pooled[g] is None:
                        pooled[g] = opool.tile([P, HIDDEN_DIM], FP, tag="pooled",
                                               name=f"pooled{g}_{rep}", bufs=2)
                    # ---- gather + transpose: xT[:, j*512 + tok] = x^T ----
                    # (bf16 copy feeds v; fp8 copy feeds the DoubleRow q/k)
                    xT = bpool.tile([P, DCH * TOK_BLK], BF, tag="xT")
                    xT8 = bpool.tile([P, DCH * TOK_BLK], F8, tag="xT8")
                    for t in range(TILES_PER_BLOCK):
                        row0 = (b * TILES_PER_BLOCK + t) * P
                        idx_sb = spool.tile([P, 1], mybir.dt.int32, tag="idx")
                        nc.sync.dma_start(out=idx_sb[:, :1],
                                          in_=idx[row0:row0 + P, None])
                        x = xpool.tile([P, INPUT_DIM], BF, tag="x")
                        nc.gpsimd.indirect_dma_start(
                            out=x[:], out_offset=None, in_=table[:],
                            in_offset=bass.IndirectOffsetOnAxis(ap=idx_sb[:, :1], axis=0),
                        )
                        pa = pspool.tile([P, INPUT_DIM], BF, tag="xp", bufs=xp_bufs)
                        for j in range(DCH):
                            nc.tensor.transpose(out=pa[:, j * P:(j + 1) * P],
                                                in_=x[:, j * P:(j + 1) * P],
                                                identity=ident[:])
                        nc.vector.tensor_copy(
                            out=xT[:].rearrange("p (j n) -> p j n", j=DCH)
                                [:, :, t * P:(t + 1) * P],
                            in_=pa[:].rearrange("p (j n) -> p j n", j=DCH),
                        )
                        nc.vector.tensor_copy(
                            out=xT8[:].rearrange("p (j n) -> p j n", j=DCH)
                                [:, :, t * P:(t + 1) * P],
                            in_=pa[:].rearrange("p (j n) -> p j n", j=DCH),
                        )

                    # ---- qT, kT: weight-stationary, N=512 tokens ----
                    # qT layout: [128 part = 2 heads x 64 d, HCH chunks x 512 tok]
                    # *_sw = partition halves swapped (for diagonal-tile scores)
                    qT = bpool.tile([P, HCH * TOK_BLK], BF, tag="qT")
                    kT = bpool.tile([P, HCH * TOK_BLK], BF, tag="kT")
                    qT_sw = bpool.tile([P, HCH * TOK_BLK], BF, tag="qTsw")
                    kT_sw = bpool.tile([P, HCH * TOK_BLK], BF, tag="kTsw")
                    w3 = lambda wsb: wsb[:].rearrange("p (j h) -> p j h", j=DCH)
                    x3 = xT8[:].rearrange("p (j n) -> p j n", j=DCH)
                    for (wsb, bsb, dst, dsw) in ((wq_sb, bq_sb, qT, qT_sw),
                                                 (wk_sb, bk_sb, kT, kT_sw)):
                        for hc in range(HCH):
                            acc = pspool.tile([P, TOK_BLK], FP, tag="acc", bufs=acc_bufs)
                            for j in range(0, DCH, 2):
                                nc.tensor.matmul(
                                    out=acc[:],
                                    lhsT=w3(wsb)[:, j:j + 2, hc * P:(hc + 1) * P],
                                    rhs=x3[:, j:j + 2, :],
                                    start=(j == 0), stop=(j == DCH - 2),
                                    perf_mode=mybir.MatmulPerfMode.DoubleRow,
                                )
                            nc.scalar.activation(
                                out=dst[:, hc * TOK_BLK:(hc + 1) * TOK_BLK],
                                in_=acc[:],
                                func=mybir.ActivationFunctionType.Identity,
                                bias=bsb[:, hc:hc + 1],
                                scale=1.0 / W8_SCALE)
                        nc.sync.dma_start(out=dsw[0:64, :], in_=dst[64:P, :])
                        nc.sync.dma_start(out=dsw[64:P, :], in_=dst[0:64, :])

                    # ---- v: x-stationary per tile; layout [128 tok, t, h, 72]
                    # with the valid-mask at [..., 64] so the ctx matmul (N=65)
                    # also produces the softmax denominator, and invalid-key
                    # rows are zeroed via the copy's per-partition scale ----
                    v = bpool.tile([P, TILES_PER_BLOCK * NUM_HEADS * 72], BF,
                                   tag="v")
                    v4 = v[:].rearrange("p (t h e) -> p t h e",
                                        t=TILES_PER_BLOCK, h=NUM_HEADS)
                    if not FULL_CTX:
                        nc.gpsimd.memset(v4[:, :, :, 64:65], 1.0)
                    mks = []
                    for t in range(TILES_PER_BLOCK):
                        row0 = (b * TILES_PER_BLOCK + t) * P
                        mk = spool.tile([P, 1], FP, tag="mk")
                        nc.sync.dma_start(out=mk[:, :1], in_=maskb[row0:row0 + P, None])
                        mks.append(mk)
                        if FULL_CTX:
                            nc.gpsimd.tensor_copy(
                                out=v4[:, t, :, 64:65],
                                in_=mk[:, None, 0:1].to_broadcast([P, NUM_HEADS, 1]))
                        acc = pspool.tile([P, HIDDEN_DIM], FP, tag="acc", bufs=acc_bufs)
                        nmm = DCH + (1 if with_v_bias else 0)
                        for j in range(DCH):
                            nc.tensor.matmul(
                                out=acc[:],
                                lhsT=xT[:, j * TOK_BLK + t * P:j * TOK_BLK + (t + 1) * P],
                                rhs=wv_sb[:, j * HIDDEN_DIM:(j + 1) * HIDDEN_DIM],
                                start=(j == 0), stop=(j == nmm - 1),
                            )
                        if with_v_bias:
                            nc.tensor.matmul(out=acc[:], lhsT=ones1[0:1, :],
                                             rhs=bv_sb[0:1, :], start=False, stop=True)
                        if FULL_CTX:
                            nc.vector.tensor_tensor(
                                out=v4[:, t, :, 0:64],
                                in0=acc[:].rearrange("p (h d) -> p h d", h=NUM_HEADS),
                                in1=mk[:, :, None].to_broadcast([P, NUM_HEADS, 64]),
                                op=mybir.AluOpType.mult)
                        else:
                            nc.scalar.activation(
                                out=v4[:, t, :, 0:64],
                                in_=acc[:].rearrange("p (h d) -> p h d",
                                                     h=NUM_HEADS),
                                func=mybir.ActivationFunctionType.Copy)

                    # ---- attention per tile (2 cells) ----
                    cns = []
                    for t in range(TILES_PER_BLOCK):
                        gt = b * TILES_PER_BLOCK + t      # global tile id
                        row0 = gt * P
                        u2_sb = spool.tile([P, 2], BF, tag="u2", bufs=5)
                        nc.sync.dma_start(out=u2_sb[:], in_=u2[row0:row0 + P, :])

                        # scores^T: [2c x 64 m, 8h x 64 l]; diagonal tiles only:
                        # head h data taken from the copy that has it at half c.
                        sc = pspool.tile([P, HIDDEN_DIM], FP, tag="att", bufs=att_bufs)
                        for h in range(NUM_HEADS):
                            hc = h // 2
                            for c in range(2):   # c inner: T0/T10 quads overlap
                                pr = slice(c * 64, c * 64 + 64)
                                kk, qq = (kT, qT) if h % 2 == c else (kT_sw, qT_sw)
                                fw = slice(hc * TOK_BLK + t * P + c * 64,
                                           hc * TOK_BLK + t * P + c * 64 + 64)
                                nc.tensor.matmul(
                                    out=sc[pr, h * 64:h * 64 + 64],
                                    lhsT=kk[pr, fw], rhs=qq[pr, fw],
                                    start=True, stop=True,
                                )
                        if FULL_CTX:
                            # exp into the diagonal (own-cell) blocks of the e
                            # ring buffer; cross-cell blocks statically zero
                            e4 = e_bufs[gt % 3]
                            for c in range(2):
                                pr = slice(c * 64, c * 64 + 64)
                                nc.scalar.activation(
                                    out=e4[pr, :, c, :],
                                    in_=sc[pr, :].rearrange("p (h l) -> p h l",
                                                            h=NUM_HEADS),
                                    func=mybir.ActivationFunctionType.Exp)
                        else:
                            e = spool.tile([P, HIDDEN_DIM], BF, tag="e")
                            nc.scalar.activation(
                                out=e[:], in_=sc[:],
                                func=mybir.ActivationFunctionType.Exp,
                                bias=mks[t][:, :1])

                        # ctx (unnormalized) + denominators fused; FULL_CTX:
                        # one matmul per head over all 128 partitions (invalid
                        # or other-cell keys contribute zero via v + ones-col
                        # masking); else per (head, cell-half). 4 heads per
                        # psum bank at 72-col pitch. Half A normalizes on DVE
                        # while half B's matmuls run.
                        cn = spool.tile([P, HIDDEN_DIM], BF, tag="cn", bufs=5)
                        for half in range(2):
                            cdh = pspool.tile([P, 512], FP, tag="cd",
                                              name=f"cd{half}", bufs=cd_bufs)
                            for hj in range(4):
                                h = half * 4 + hj
                                col = 72 * hj
                                if FULL_CTX:
                                    nc.tensor.matmul(
                                        out=cdh[:, col:col + 65],
                                        lhsT=e4[:, h, :, :],
                                        rhs=v4[:, t, h, 0:65],
                                        start=True, stop=True,
                                    )
                                else:
                                    for c in range(2):
                                        pr = slice(c * 64, c * 64 + 64)
                                        nc.tensor.matmul(
                                            out=cdh[pr, col:col + 65],
                                            lhsT=e[pr, h * 64:h * 64 + 64],
                                            rhs=v4[pr, t, h, 0:65],
                                            start=True, stop=True,
                                        )
                            cdv = cdh[:, 0:288].rearrange("p (j s) -> p j s", j=4)
                            r = spool.tile([P, 4], FP, tag="r")
                            nc.vector.reciprocal(out=r[:], in_=cdv[:, :, 64])
                            nc.vector.tensor_tensor(
                                out=cn[:, half * 256:(half + 1) * 256]
                                    .rearrange("p (j d) -> p j d", j=4),
                                in0=cdv[:, :, 0:64],
                                in1=r[:, :, None].to_broadcast([P, 4, HEAD_DIM]),
                                op=mybir.AluOpType.mult,
                            )
                        cns.append((gt, cn, u2_sb))

                    # pooled rows after the block (keeps PE off the DVE
                    # normalize critical path): pooled[2 cells, :] = u2^T @ cn
                    for (gt, cn, u2_sb) in cns:
                        tl = gt - g * (TILES_PER_CORE // CELL_GROUPS)
                        bp = pspool.tile([2, HIDDEN_DIM], FP, tag="att", bufs=att_bufs)
                        nc.tensor.matmul(
                            out=bp[0:2, :],
                            lhsT=u2_sb[:, 0:2], rhs=cn[:],
                            start=True, stop=True,
                        )
                        bp_sb = spool.tile([2, HIDDEN_DIM], FP, tag="bps")
                        nc.scalar.activation(
                            out=bp_sb[0:2, :], in_=bp[0:2, :],
                            func=mybir.ActivationFunctionType.Copy)
                        nc.sync.dma_start(
                            out=pooled[g][2 * tl:2 * tl + 2, :], in_=bp_sb[0:2, :])

                # ---- final projection per group of 128 cells ----
                for g in range(CELL_GROUPS):
                    pt = pspool.tile([P, HIDDEN_DIM], FP, tag="xp", bufs=xp_bufs)
                    for hc in range(HCH):
                        nc.tensor.transpose(out=pt[:, hc * P:(hc + 1) * P],
                                            in_=pooled[g][:, hc * P:(hc + 1) * P],
                                            identity=ident_f[:])
                    ptsb = opool.tile([P, HIDDEN_DIM], BF, tag="ptsb")
                    nc.vector.tensor_copy(out=ptsb[:], in_=pt[:])
                    acc = pspool.tile([P, OUTPUT_DIM], FP, tag="acc", bufs=acc_bufs)
                    for hc in range(HCH):
                        nc.tensor.matmul(
                            out=acc[:], lhsT=ptsb[:, hc * P:(hc + 1) * P],
                            rhs=wf_sb[:, hc * OUTPUT_DIM:(hc + 1) * OUTPUT_DIM],
                            start=(hc == 0), stop=(hc == HCH - 1),
                        )
                    osb = opool.tile([P, OUTPUT_DIM], FP, tag="osb")
                    nc.scalar.activation(out=osb[:], in_=acc[:],
                                         func=mybir.ActivationFunctionType.Copy)
                    nc.sync.dma_start(out=out[g * P:(g + 1) * P, :], in_=osb[:])

    nc.compile()
    return nc


def preprocess(chunk_features, Wq, bq, Wk, bk, Wv, bv, W_in, b_in, Wo, bo,
               Wout, bout, cell_idx, cell_len):
    """Host-side weight folding + per-core input maps. Returns (in_maps, b_final,
    with_v_bias)."""
    import ml_dtypes
    f32 = np.float32
    bf16 = ml_dtypes.bfloat16
    f8 = mybir.dt.np(F8)
    cf = np.ascontiguousarray(np.asarray(chunk_features, f32).astype(bf16))
    Wq, Wk, Wv = (np.asarray(w, f32) for w in (Wq, Wk, Wv))
    bq, bk, bv = (np.asarray(x, f32) for x in (bq, bk, bv))
    W_in = np.asarray(W_in, f32)
    b_in = np.asarray(b_in, f32)
    Wo, bo = np.asarray(Wo, f32), np.asarray(bo, f32)
    Wout, bout = np.asarray(Wout, f32), np.asarray(bout, f32)

    Wiq, Wik, Wiv = np.split(W_in, 3, axis=0)
    biq, bik, biv = np.split(b_in, 3)
    scale = f32(1.0 / np.sqrt(HEAD_DIM))
    wq_eff = (Wiq @ Wq) * scale          # [512, 768]
    wk_eff = Wik @ Wk
    wv_eff = Wiv @ Wv
    bq_eff = (Wiq @ bq + biq) * scale    # [512]
    bk_eff = Wik @ bk + bik
    bv_eff = Wiv @ bv + biv
    wfin = Wout @ Wo                     # [256, 512]
    b_final = bo @ Wout.T + bout         # [256]

    wq_t = np.ascontiguousarray((wq_eff.T * W8_SCALE).astype(f8))  # [768, 512]
    wk_t = np.ascontiguousarray((wk_eff.T * W8_SCALE).astype(f8))
    wv_t = np.ascontiguousarray(wv_eff.T.astype(bf16))
    wf_t = np.ascontiguousarray(wfin.T.astype(bf16))     # [512, 256]
    bq_c = np.ascontiguousarray(bq_eff.reshape(HCH, P).T)  # [128, 4] f32
    bk_c = np.ascontiguousarray(bk_eff.reshape(HCH, P).T)
    bv_r = np.ascontiguousarray(bv_eff.reshape(1, HIDDEN_DIM).astype(bf16))
    with_v_bias = bool(np.any(bv_eff != 0))

    ci = np.asarray(cell_idx).astype(np.int32)             # [2048, 64]
    ln = np.maximum(np.asarray(cell_len).astype(np.int64), 1)
    ln = np.minimum(ln, MAX_LEN).astype(np.int32)          # [2048]
    pos = np.arange(MAX_LEN, dtype=np.int32)
    valid = pos[None, :] < ln[:, None]                     # [2048, 64]
    if FULL_CTX:
        maskb_full = np.where(valid, f32(1.0), f32(0.0))   # [2048, 64]
    else:
        maskb_full = np.where(valid, f32(0.0), f32(-1e30))
    u_full = (valid / ln[:, None]).astype(f32)             # [2048, 64]

    in_maps = []
    for core in range(N_CORES):
        cs = slice(core * CELLS_PER_CORE, (core + 1) * CELLS_PER_CORE)
        idx_c = np.ascontiguousarray(ci[cs].reshape(-1))
        mb_c = np.ascontiguousarray(maskb_full[cs].reshape(-1))
        u_c = u_full[cs]                                   # [256, 64]
        u2_c = np.zeros((TILES_PER_CORE, P, 2), f32)
        u2_c[:, 0:64, 0] = u_c[0::2]
        u2_c[:, 64:128, 1] = u_c[1::2]
        in_maps.append({
            "table": cf,
            "wq_t": wq_t, "wk_t": wk_t, "wv_t": wv_t, "wf_t": wf_t,
            "bq_c": bq_c, "bk_c": bk_c, "bv_r": bv_r,
            "idx": idx_c, "maskb": mb_c,
            "u2": u2_c.reshape(TILES_PER_CORE * P, 2).astype(bf16),
        })
    return in_maps, b_final, with_v_bias


_NC_CACHE: dict = {}


def get_nc(with_v_bias: bool, repeat: int = 1):
    key = (with_v_bias, repeat)
    if key not in _NC_CACHE:
        _NC_CACHE[key] = build_kernel(with_v_bias, repeat=repeat)
    return _NC_CACHE[key]


def kernel(**inputs) -> np.ndarray:
    in_maps, b_final, with_v_bias = preprocess(**inputs)
    nc = get_nc(with_v_bias)
    res = run_bass_kernel_spmd(nc, in_maps, list(range(N_CORES)))
    out = np.concatenate([res.results[i]["out"] for i in range(N_CORES)], axis=0)
    return (out + b_final[None, :]).astype(np.float32)
